# revision 24
# baseline (speedup 1.0000x reference)
"""GQA attention kernel for Trainium2, 8 NeuronCores (bf16 compute).

Problem: B=1, S=4096, HIDDEN=2048, 8 query heads x d=256, 1 shared KV head,
causal mask, fp32 I/O.

Sharding: head-parallel attention with seq-split projections.
Per core j (owning head h=j and row block rows[512j:512j+512]):
  1. load xT_own [2048, 512] (host pre-transposed, bf16)
  2. kv proj (own rows) -> kvT_own bf16 -> AllGather kvT_all [2048, 512]
  3. q proj (own rows, all heads) -> qT_own bf16 -> AllToAll ->
     qT_h [256, 4096] (own head, all rows)
  4. values kv_sb produced locally by 4 wide DMA-transposes of kvT_all
     (issued before the q AllToAll so they only depend on the AllGather)
  5. causal flash attention, software-pipelined so the PE never waits on
     the scalar engine's exp: scores of key-group kg+1 issue before the AV
     matmuls of kg. The causal mask folds into the score PSUM accumulation
     as one extra identity@mask matmul (no vector dependency), and the
     softmax denominator partials accumulate on the vector engine
     (p_acc += P tile) so the PE spends no per-group slots on them.
  6. normalize, AllToAll -> attnout^T all heads for own rows [2048, 512]
  7. output projection (own rows) + bo -> out [512, 2048] fp32
Host concatenates the 8 row blocks.

Per-hs weight tiles avoid whole-tile false dependencies (a tile written by
16 DMAs blocks its first reader until the last DMA lands); constants load
after the AllGather trigger so its quiesce fires early.
"""

import sys

import numpy as np

sys.path.insert(0, "/opt/trn_rl_repo")

S = 4096
HID = 2048
NH = 8
D = 256
NCORES = 8
R = S // NCORES  # 512 rows per core
NEG = -1.0e9
SCALE = 1.0 / 16.0  # 1/sqrt(256)

_BUILT = None


def _build():
    global _BUILT
    if _BUILT is not None:
        return _BUILT

    from contextlib import ExitStack

    from concourse import bacc, tile
    from concourse.bass import mybir

    dt = mybir.dt
    f32 = dt.float32
    f32r = dt.float32r
    bf16 = dt.bfloat16
    AF = mybir.ActivationFunctionType

    nc = bacc.Bacc(
        "TRN2",
        target_bir_lowering=False,
        debug=False,
        num_devices=NCORES,
    )

    # ---- DRAM I/O ----
    xT_own = nc.dram_tensor("xT_own", [HID, R], bf16, kind="ExternalInput")
    wq2d = nc.dram_tensor("wq2d", [HID, HID], bf16, kind="ExternalInput")
    bq_col = nc.dram_tensor("bq_col", [HID, 1], f32, kind="ExternalInput")
    wkv2d = nc.dram_tensor("wkv2d", [HID, D], bf16, kind="ExternalInput")
    bkv_col = nc.dram_tensor("bkv_col", [D, 1], f32, kind="ExternalInput")
    wo2d = nc.dram_tensor("wo2d", [HID, HID], bf16, kind="ExternalInput")
    bo_row = nc.dram_tensor("bo_row", [1, HID], f32, kind="ExternalInput")
    out = nc.dram_tensor("out", [R, HID], f32, kind="ExternalOutput")

    # ---- internal DRAM (collective buffers) ----
    grp = [list(range(NCORES))]
    qT_send = nc.dram_tensor("qT_send", [HID, R], bf16)
    qT_recv = nc.dram_tensor("qT_recv", [HID, R], bf16)
    kvT_send = nc.dram_tensor("kvT_send", [D, R], bf16)
    kvT_all = nc.dram_tensor("kvT_all", [NCORES * D, R], bf16, addr_space="Shared")
    ao_send = nc.dram_tensor("ao_send", [HID, R], bf16)
    ao_recv = nc.dram_tensor("ao_recv", [HID, R], bf16)

    # ---- compile-time constants (embedded in NEFF) ----
    ones_col_np = np.ones((128, 1), dtype=np.float32)
    ident_np = np.eye(128, dtype=np.float32)
    # diagonal masks for a 512-row q block vs its two 256-key diagonal groups
    # layout [128 keys, 2 groups * 2 slices * 512 rows]
    mask_np = np.empty((128, 2048), dtype=np.float32)
    kappa = np.arange(128)[:, None]
    rows = np.arange(512)[None, :]
    for grel in range(2):
        for sl in range(2):
            keyrel = 256 * grel + 128 * sl + kappa
            blk = np.where(keyrel <= rows, 0.0, NEG).astype(np.float32)
            mask_np[:, 1024 * grel + 512 * sl : 1024 * grel + 512 * sl + 512] = blk
    ones_col_d = nc.inline_tensor(ones_col_np, "ones_col")
    ident_d = nc.inline_tensor(ident_np, "ident_f")
    mask_d = nc.inline_tensor(mask_np, "mask_const")

    def r32(ap):
        return ap.bitcast(f32r)

    with tile.TileContext(nc) as tc:
        with ExitStack() as top:
            cpool = top.enter_context(tc.tile_pool(name="const", bufs=1))
            attn_pool = top.enter_context(tc.tile_pool(name="attn", bufs=1))
            qTb = [
                attn_pool.tile([128, 2 * R], bf16, tag=f"qTb{b}", name=f"qTb{b}")
                for b in range(8)
            ]
            kvT = attn_pool.tile([128, 2 * S], bf16, tag="kvT")
            # kv_sb[kappa, 2048*m + 256*s + d]: keys 512*s + 128*m + kappa
            kv_sb = attn_pool.tile([128, 32 * D], bf16, tag="kv_sb")

            with ExitStack() as ph123:
                w_pool = ph123.enter_context(tc.tile_pool(name="win", bufs=1))
                xTs = [
                    w_pool.tile([128, R], bf16, tag=f"xTs{hs}", name=f"xTs{hs}")
                    for hs in range(16)
                ]
                for hs in range(16):
                    nc.sync.dma_start(
                        xTs[hs][:], xT_own[128 * hs : 128 * hs + 128, :]
                    )
                wkvs = [
                    w_pool.tile([128, D], bf16, tag=f"wkvs{hs}", name=f"wkvs{hs}")
                    for hs in range(16)
                ]
                for hs in range(16):
                    nc.scalar.dma_start(
                        wkvs[hs][:], wkv2d[128 * hs : 128 * hs + 128, :]
                    )
                bkv_sb = cpool.tile([128, 2], f32, tag="bkv")
                for dh in range(2):
                    nc.sync.dma_start(
                        bkv_sb[:, dh : dh + 1], bkv_col[128 * dh : 128 * dh + 128, :]
                    )

                # ============ phase 2: kv projection + AllGather ============
                with ExitStack() as ph2:
                    kv_psum = ph2.enter_context(
                        tc.tile_pool(name="kv_psum", bufs=2, space="PSUM")
                    )
                    kv_out = ph2.enter_context(tc.tile_pool(name="kv_out", bufs=2))
                    for dh in range(2):
                        ps = kv_psum.tile([128, R], f32, tag="kvps")
                        for hs in range(16):
                            nc.tensor.matmul(
                                ps[:],
                                wkvs[hs][:, 128 * dh : 128 * dh + 128],
                                xTs[hs][:],
                                start=(hs == 0),
                                stop=(hs == 15),
                            )
                        kvt_sb = kv_out.tile([128, R], bf16, tag="kvt")
                        nc.scalar.activation(
                            kvt_sb[:], ps[:], AF.Identity, bias=bkv_sb[:, dh : dh + 1]
                        )
                        nc.scalar.dma_start(
                            kvT_send[128 * dh : 128 * dh + 128, :], kvt_sb[:]
                        )
                    nc.gpsimd.collective_compute(
                        "AllGather",
                        mybir.AluOpType.bypass,
                        replica_groups=grp,
                        ins=[kvT_send[:]],
                        outs=[kvT_all[:]],
                    )

                # wq streams after the AllGather call so the collective's
                # quiesce never waits on the 8MB weight stream; nothing
                # AllGather-gated precedes it on either queue
                wqs = [
                    w_pool.tile([128, HID], bf16, tag=f"wqs{hs}", name=f"wqs{hs}")
                    for hs in range(16)
                ]
                for hs in range(16):
                    eng = nc.sync if hs % 2 == 0 else nc.scalar
                    for half in range(2):
                        eng.dma_start(
                            wqs[hs][:, 1024 * half : 1024 * half + 1024],
                            wq2d[
                                128 * hs : 128 * hs + 128,
                                1024 * half : 1024 * half + 1024,
                            ],
                        )
                bq_sb = cpool.tile([128, 16], f32, tag="bq")
                for gd in range(16):
                    nc.scalar.dma_start(
                        bq_sb[:, gd : gd + 1], bq_col[128 * gd : 128 * gd + 128, :]
                    )
                bor_sb = cpool.tile([1, HID], f32, tag="bor")
                nc.scalar.dma_start(bor_sb[:], bo_row[:])
                # constants (needed later; kept off the pre-AllGather queues)
                ones_col_f = cpool.tile([128, 1], f32, tag="ones_col_f")
                nc.sync.dma_start(ones_col_f[:], ones_col_d[:])
                ident_f = cpool.tile([128, 128], f32, tag="ident_f")
                nc.sync.dma_start(ident_f[:], ident_d[:])
                ident_bf = cpool.tile([128, 128], bf16, tag="ident_bf")
                nc.vector.tensor_copy(ident_bf[:], ident_f[:])
                mask_f = cpool.tile([128, 2048], f32, tag="mask_f")
                nc.sync.dma_start(mask_f[:], mask_d[:])
                mask_bf = cpool.tile([128, 2048], bf16, tag="mask_bf")
                nc.vector.tensor_copy(mask_bf[:], mask_f[:])
                # attention kv operands: tail of both queues behind wq;
                # they wait on the AllGather and drain during q proj
                for src_ in range(8):
                    for dh in range(2):
                        nc.sync.dma_start(
                            kvT[:, S * dh + R * src_ : S * dh + R * src_ + R],
                            kvT_all[D * src_ + 128 * dh : D * src_ + 128 * dh + 128, :],
                        )

                # ============ phase 3: q projection + AllToAll ============
                with ExitStack() as ph3:
                    q_psum = ph3.enter_context(
                        tc.tile_pool(name="q_psum", bufs=8, space="PSUM")
                    )
                    q_out = ph3.enter_context(tc.tile_pool(name="q_out", bufs=1))
                    # two waves of 8; hs-outer so matmuls consume wq as it streams
                    q_sbs = []
                    for wave in range(2):
                        pss = [
                            q_psum.tile([128, R], f32, tag="qps", name=f"qps{wave}_{i}")
                            for i in range(8)
                        ]
                        for hs in range(16):
                            for i in range(8):
                                gd = 8 * wave + i
                                nc.tensor.matmul(
                                    pss[i][:],
                                    wqs[hs][:, 128 * gd : 128 * gd + 128],
                                    xTs[hs][:],
                                    start=(hs == 0),
                                    stop=(hs == 15),
                                )
                        for i in range(8):
                            gd = 8 * wave + i
                            q_sb = q_out.tile(
                                [128, R], bf16, tag=f"qsb{gd}", name=f"qsb{gd}"
                            )
                            nc.scalar.activation(
                                q_sb[:],
                                pss[i][:],
                                AF.Identity,
                                bias=bq_sb[:, gd : gd + 1],
                            )
                            q_sbs.append(q_sb)
                    # all 16 stores after BOTH waves' activations: DMA-ring
                    # stalls on the stores must not delay the activations
                    # that release PSUM for wave B
                    for gd in range(16):
                        nc.scalar.dma_start(
                            qT_send[128 * gd : 128 * gd + 128, :], q_sbs[gd][:]
                        )
                # transpose kv values on the PE (kvT SBUF slices -> PSUM ->
                # kv_sb). Replaces XBAR DMA-transposes: runs ~7us on the PE
                # right after q proj and keeps the AllToAll quiesce free of
                # slow transpose DMAs
                with ExitStack() as pht:
                    tp_psum = pht.enter_context(
                        tc.tile_pool(name="tp_psum", bufs=4, space="PSUM")
                    )
                    for k in range(32):
                        for dh in range(2):
                            tp = tp_psum.tile([128, 128], bf16, tag="tp")
                            nc.tensor.transpose(
                                tp[:],
                                kvT[:, S * dh + 128 * k : S * dh + 128 * k + 128],
                                ident_bf[:],
                            )
                            dst = kv_sb[
                                :,
                                2048 * (k % 4)
                                + 256 * (k // 4)
                                + 128 * dh : 2048 * (k % 4)
                                + 256 * (k // 4)
                                + 128 * dh
                                + 128,
                            ]
                            if (2 * k + dh) % 2 == 0:
                                nc.vector.tensor_copy(dst, tp[:])
                            else:
                                nc.scalar.copy(dst, tp[:])
                nc.gpsimd.collective_compute(
                    "AllToAll",
                    mybir.AluOpType.bypass,
                    replica_groups=grp,
                    ins=[qT_send[:]],
                    outs=[qT_recv[:]],
                )

            # ============ phase 4/5: attention ============
            with ExitStack() as ph45:
                wo_pool = ph45.enter_context(tc.tile_pool(name="wo", bufs=1))
                wo_sb = wo_pool.tile([128, 16 * HID], bf16, tag="wo_sb")

                with ExitStack() as ph5:
                    for src in range(8):
                        for dh in range(2):
                            nc.sync.dma_start(
                                qTb[src][:, R * dh : R * dh + R],
                                qT_recv[D * src + 128 * dh : D * src + 128 * dh + 128, :],
                            )
                    # wo prefetch drains during early attention compute
                    for k in range(16):
                        nc.sync.dma_start(
                            wo_sb[:, HID * k : HID * k + HID],
                            wo2d[128 * k : 128 * k + 128, :],
                        )

                    s_psum = ph5.enter_context(
                        tc.tile_pool(name="s_psum", bufs=2, space="PSUM")
                    )
                    ao_psum = ph5.enter_context(
                        tc.tile_pool(name="ao_psum", bufs=2, space="PSUM")
                    )
                    den_psum = ph5.enter_context(
                        tc.tile_pool(name="den_psum", bufs=2, space="PSUM")
                    )
                    p_pool = ph5.enter_context(tc.tile_pool(name="p", bufs=4))
                    pa_pool = ph5.enter_context(tc.tile_pool(name="pa", bufs=2))
                    nrm_pool = ph5.enter_context(tc.tile_pool(name="nrm", bufs=2))
                    aon_pool = ph5.enter_context(tc.tile_pool(name="aon", bufs=4))

                    # scores+mask+exp for one key group; mask folds into the
                    # PSUM accumulation via an identity@mask matmul
                    def score_block(b, kg):
                        st = s_psum.tile([128, 1024], f32, tag="st")
                        diag = kg >= 2 * b
                        grel = kg - 2 * b
                        for sl in range(2):
                            k = 2 * kg + sl
                            for dh in range(2):
                                nc.tensor.matmul(
                                    st[:, 512 * sl : 512 * sl + 512],
                                    kvT[:, S * dh + 128 * k : S * dh + 128 * k + 128],
                                    qTb[b][:, R * dh : R * dh + R],
                                    start=(dh == 0),
                                    stop=(dh == 1) and not diag,
                                )
                            if diag:
                                nc.tensor.matmul(
                                    st[:, 512 * sl : 512 * sl + 512],
                                    ident_bf[:],
                                    mask_bf[:, 1024 * grel + 512 * sl : 1024 * grel + 512 * sl + 512],
                                    start=False,
                                    stop=True,
                                )
                        pt = p_pool.tile([128, 1024], bf16, tag="pt")
                        nc.scalar.activation(pt[:], st[:], AF.Exp, scale=SCALE)
                        return pt

                    pending = score_block(0, 0)
                    for b in range(8):
                        aops = [
                            ao_psum.tile([128, R], f32, tag="aops", name=f"aops{b}_{i}")
                            for i in range(2)
                        ]
                        p_acc = pa_pool.tile([128, 1024], f32, tag="p_acc")
                        ngroups = 2 * (b + 1)
                        for kg in range(ngroups):
                            pt = pending
                            if kg + 1 < ngroups:
                                pending = score_block(b, kg + 1)
                            elif b + 1 < 8:
                                pending = score_block(b + 1, 0)
                            else:
                                pending = None
                            for sl in range(2):
                                k = 2 * kg + sl
                                for dh in range(2):
                                    nc.tensor.matmul(
                                        aops[dh][:],
                                        kv_sb[
                                            :,
                                            2048 * (k % 4)
                                            + 256 * (k // 4)
                                            + 128 * dh : 2048 * (k % 4)
                                            + 256 * (k // 4)
                                            + 128 * dh
                                            + 128,
                                        ],
                                        pt[:, 512 * sl : 512 * sl + 512],
                                        start=(kg == 0 and sl == 0),
                                        stop=(kg == ngroups - 1 and sl == 1),
                                    )
                            # denominator partials accumulate on the vector
                            # engine; the PE spends no per-group slots on them
                            if kg == 0:
                                nc.vector.tensor_copy(r32(p_acc[:]), pt[:])
                            else:
                                nc.vector.tensor_add(
                                    r32(p_acc[:]), r32(p_acc[:]), pt[:]
                                )
                        # reduce p_acc over keys with two [1,512] matmuls
                        denp = den_psum.tile([1, R], f32, tag="denp")
                        nc.tensor.matmul(
                            denp[:],
                            r32(ones_col_f[:]),
                            r32(p_acc[:, 0:512]),
                            start=True,
                            stop=False,
                        )
                        nc.tensor.matmul(
                            denp[:],
                            r32(ones_col_f[:]),
                            r32(p_acc[:, 512:1024]),
                            start=False,
                            stop=True,
                        )
                        den_sb = nrm_pool.tile([1, R], f32, tag="den_sb")
                        nc.vector.reciprocal_approx_fast(den_sb[:], denp[:])
                        bc = nrm_pool.tile([128, R], f32, tag="bc")
                        nc.gpsimd.partition_broadcast(bc[:], den_sb[:])
                        for dh in range(2):
                            aon = aon_pool.tile([128, R], bf16, tag="aon")
                            nc.vector.tensor_mul(aon[:], aops[dh][:], bc[:])
                            nc.sync.dma_start(
                                ao_send[D * b + 128 * dh : D * b + 128 * dh + 128, :],
                                aon[:],
                            )
                    nc.gpsimd.collective_compute(
                        "AllToAll",
                        mybir.AluOpType.bypass,
                        replica_groups=grp,
                        ins=[ao_send[:]],
                        outs=[ao_recv[:]],
                    )

                # ============ phase 6: output projection ============
                with ExitStack() as ph6:
                    bor_bc = cpool.tile([128, HID], f32, tag="bor_bc")
                    nc.gpsimd.partition_broadcast(bor_bc[:], bor_sb[:])
                    o_in = ph6.enter_context(tc.tile_pool(name="o_in", bufs=1))
                    aoTs = [
                        o_in.tile([128, R], bf16, tag=f"aoT{k}", name=f"aoT{k}")
                        for k in range(16)
                    ]
                    for k in range(16):
                        nc.sync.dma_start(
                            aoTs[k][:],
                            ao_recv[128 * k : 128 * k + 128, :],
                        )
                    o_psum = ph6.enter_context(
                        tc.tile_pool(name="o_psum", bufs=4, space="PSUM")
                    )
                    o_out = ph6.enter_context(tc.tile_pool(name="o_out", bufs=2))
                    for rc in range(4):
                        osb = o_out.tile([128, HID], f32, tag="osb")
                        for ncol in range(4):
                            ps = o_psum.tile([128, 512], f32, tag="ops")
                            for k in range(16):
                                nc.tensor.matmul(
                                    ps[:],
                                    aoTs[k][:, 128 * rc : 128 * rc + 128],
                                    wo_sb[:, HID * k + 512 * ncol : HID * k + 512 * ncol + 512],
                                    start=(k == 0),
                                    stop=(k == 15),
                                )
                            nc.vector.tensor_add(
                                osb[:, 512 * ncol : 512 * ncol + 512],
                                ps[:],
                                bor_bc[:, 512 * ncol : 512 * ncol + 512],
                            )
                        nc.sync.dma_start(out[128 * rc : 128 * rc + 128, :], osb[:])

    nc.compile()
    _BUILT = nc
    return nc


def _make_in_maps(x, wq, bq, wkv, bkv, wo, bo):
    import ml_dtypes

    bf = ml_dtypes.bfloat16
    x = np.asarray(x, dtype=np.float32).reshape(S, HID).astype(bf)
    shared = {
        "wq2d": np.ascontiguousarray(
            np.asarray(wq, dtype=np.float32).reshape(HID, HID).astype(bf)
        ),
        "bq_col": np.ascontiguousarray(
            np.asarray(bq, dtype=np.float32).reshape(HID, 1)
        ),
        "wkv2d": np.ascontiguousarray(
            np.asarray(wkv, dtype=np.float32).reshape(HID, D).astype(bf)
        ),
        "bkv_col": np.ascontiguousarray(
            np.asarray(bkv, dtype=np.float32).reshape(D, 1)
        ),
        "wo2d": np.ascontiguousarray(
            np.asarray(wo, dtype=np.float32).reshape(HID, HID).astype(bf)
        ),
        "bo_row": np.ascontiguousarray(
            np.asarray(bo, dtype=np.float32).reshape(1, HID)
        ),
    }
    in_maps = []
    for j in range(NCORES):
        m = dict(shared)
        m["xT_own"] = np.ascontiguousarray(x[R * j : R * j + R, :].T)
        in_maps.append(m)
    return in_maps


def _run(inputs, trace=False, **trace_kwargs):
    from concourse.bass_utils import run_bass_kernel_spmd

    nc = _build()
    in_maps = _make_in_maps(
        inputs["x"],
        inputs["wq"],
        inputs["bq"],
        inputs["wkv"],
        inputs["bkv"],
        inputs["wo"],
        inputs["bo"],
    )
    res = run_bass_kernel_spmd(
        nc, in_maps, list(range(NCORES)), trace=trace, **trace_kwargs
    )
    outs = [np.asarray(res.results[j]["out"]) for j in range(NCORES)]
    full = np.concatenate(outs, axis=0).reshape(1, S, HID).astype(np.float32)
    return full, res


def kernel(**inputs):
    full, _ = _run(inputs, trace=False)
    return full


if __name__ == "__main__":
    rng = np.random.default_rng(0)
    ins = {
        "x": rng.standard_normal((1, S, HID), dtype=np.float32),
        "wq": rng.standard_normal((HID, NH, D), dtype=np.float32) / 45.25,
        "bq": np.zeros((NH, D), np.float32),
        "bkv": np.zeros((1, D), np.float32),
        "wkv": rng.standard_normal((HID, 1, D), dtype=np.float32) / 45.25,
        "wo": rng.standard_normal((NH, D, HID), dtype=np.float32) / 45.25,
        "bo": np.zeros((HID,), np.float32),
        "mask": np.tril(np.ones((S, S), bool))[None, None],
    }
    out = kernel(**ins)
    print("out", out.shape, out.dtype, float(np.abs(out).max()))
